# revision 2
# baseline (speedup 1.0000x reference)
"""ChemConv Bass kernel for 8 TRN2 NeuronCores.

Reference math:
    node_connection[a,f,i] = sum_n conn[a,n,f] * x[n,i]
    bond_score[a,o,f]      = sum_i node_connection[a,f,i] * pf[o,f,i]
    out[a,o] = sum_f bond_score[a,o,f]*bf[o,f,0] + sum_{f,c} bp[a,f,c]*bf[o,f,1+c]
collapses to
    out[a,o] = sum_{n,f} conn[a,n,f] * Y[n,f,o] + bond[a,o]
    Y[n,f,o] = sum_i x[n,i] * W[o,f,i],   W = pf * bf[:,:,0:1]

Device does the single large contraction (conn is 201MB, the memory-bound
stream); Y (3MB) and the bond term (0.5MB) are precomputed on the host and
the bond term is added host-side after the gather.

Sharding: cores 2p and 2p+1 share atom slab p (512 atoms); core 2p
contracts n-blocks 0..7, core 2p+1 blocks 8..15 (k-split).  The host sums
each pair's partial outputs.  This shape lets every matmul be the
max-size instruction [K=128]x[M=64]x[N=512] (PSUM bank caps N*4B at 2KB),
so each core runs only 96 matmuls in one PSUM accumulation group - the
kernel is bound by per-instruction cost, so instruction count is the
metric that matters (~103/core vs ~420 for the naive schedule).

conn is cast to bf16 host-side (halves HBM bytes; measured output rel-err
~2e-3 vs the 2e-2 gate) and laid out per-partition-contiguous
[128, 96*512] so each DMA slice moves 24KB-contiguous runs per partition
instead of 1KB strided pieces.
"""

import numpy as np
import ml_dtypes

import concourse.bass as bass
import concourse.tile as tile
from concourse import bacc, mybir
from concourse.bass_utils import run_bass_kernel_spmd

A = 2048
IN_DEPTH = 64
OUT_DEPTH = 64
F = 12
NCORES = 8
NPAIR = NCORES // 2
ASLAB = A // NPAIR        # 512 atoms per core pair
KP = 128
NBLK = A // KP            # 16 n-blocks total
NBH = NBLK // 2           # 8 n-blocks per core (k-split)
JC = NBH * F              # 96 contraction chunks per core
YW = F * OUT_DEPTH        # 768

BF16 = mybir.dt.bfloat16
F32 = mybir.dt.float32
NPBF16 = ml_dtypes.bfloat16

_cache = {}


def _build_nc(repeat=1, slices=4, conn_kind="ExternalInput"):
    assert JC % slices == 0
    CPS = JC // slices
    CW = CPS * ASLAB

    nc = bacc.Bacc("TRN2", target_bir_lowering=False, debug=False)

    connb = nc.dram_tensor("connb", [KP, JC * ASLAB], BF16, kind=conn_kind).ap()
    yb = nc.dram_tensor("yb", [KP, JC * OUT_DEPTH], BF16,
                        kind="ExternalInput").ap()
    out_t = nc.dram_tensor("out_t", [OUT_DEPTH, ASLAB], F32,
                           kind="ExternalOutput").ap()

    with tile.TileContext(nc) as tc:
        with (
            tc.tile_pool(name="const", bufs=1) as cpool,
            tc.tile_pool(name="conn", bufs=1) as connpool,
            tc.tile_pool(name="osb", bufs=2) as opool,
            tc.tile_pool(name="acc", bufs=2, space="PSUM") as apool,
        ):
            # yb on the ACT ring so the SP ring belongs to the conn stream
            y_sb = cpool.tile([KP, JC * OUT_DEPTH], BF16)
            nc.scalar.dma_start(y_sb[:], yb[:])

            for rep in range(repeat):
                csb = connpool.tile([KP, JC * ASLAB], BF16, tag="conn",
                                    name=f"conn_{rep}")
                for s in range(slices):
                    nc.sync.dma_start(csb[:, s * CW:(s + 1) * CW],
                                      connb[:, s * CW:(s + 1) * CW])

                acc = apool.tile([OUT_DEPTH, ASLAB], F32, tag="acc")
                for j in range(JC):
                    nc.tensor.matmul(
                        acc[:],
                        y_sb[:, j * OUT_DEPTH:(j + 1) * OUT_DEPTH],
                        csb[:, j * ASLAB:(j + 1) * ASLAB],
                        start=(j == 0), stop=(j == JC - 1),
                    )

                out_sb = opool.tile([OUT_DEPTH, ASLAB], F32, tag="osb")
                nc.vector.tensor_copy(out_sb[:], acc[:])
                nc.sync.dma_start(out_t[:], out_sb[:])

    nc.compile()
    return nc


def _prep(node_property_tensor, connectivity_tensor, bond_property_tensor,
          property_filters, bond_filters):
    x = np.asarray(node_property_tensor, dtype=np.float32)
    conn = np.asarray(connectivity_tensor, dtype=np.float32)
    pf = np.asarray(property_filters, dtype=np.float32)
    bf = np.asarray(bond_filters, dtype=np.float32)

    W = pf * bf[:, :, 0:1]                                   # (O, F, I)
    wr = W.transpose(2, 1, 0).reshape(IN_DEPTH, YW)          # (I, F*O)
    Y = (x @ wr).astype(NPBF16)                              # (A, F*O)

    conn_bf = conn.astype(NPBF16)                            # (A, A, F)

    in_maps = []
    for c in range(NCORES):
        p, half = divmod(c, 2)
        a0 = p * ASLAB
        nb0 = half * NBH
        # conn[a, n, f] -> [p=n%128, nb_local, f, a] -> [128, JC*ASLAB]
        sub = conn_bf[a0:a0 + ASLAB, nb0 * KP:(nb0 + NBH) * KP, :]
        connH = np.ascontiguousarray(
            sub.reshape(ASLAB, NBH, KP, F).transpose(2, 1, 3, 0)
        ).reshape(KP, JC * ASLAB)
        # Y[n, f*64+o] -> [p, nb_local, f, o] -> [128, JC*64]
        ybH = np.ascontiguousarray(
            Y[nb0 * KP:(nb0 + NBH) * KP].reshape(NBH, KP, F, OUT_DEPTH)
            .transpose(1, 0, 2, 3)
        ).reshape(KP, JC * OUT_DEPTH)
        in_maps.append({"connb": connH, "yb": ybH})
    return in_maps


def kernel(node_property_tensor, connectivity_tensor, bond_property_tensor,
           property_filters, bond_filters):
    bp = np.asarray(bond_property_tensor, dtype=np.float32)
    bf = np.asarray(bond_filters, dtype=np.float32)

    in_maps = _prep(node_property_tensor, connectivity_tensor,
                    bond_property_tensor, property_filters, bond_filters)

    if "nc" not in _cache:
        _cache["nc"] = _build_nc()
    nc = _cache["nc"]

    res = run_bass_kernel_spmd(nc, in_maps, core_ids=list(range(NCORES)))

    # host: sum the two k-half partials per slab, add the (tiny) bond term
    bond = np.einsum('afc,ofc->ao', bp, bf[:, :, 1:3]).astype(np.float32)
    out = np.empty((A, OUT_DEPTH), dtype=np.float32)
    for p in range(NPAIR):
        part = res.results[2 * p]["out_t"] + res.results[2 * p + 1]["out_t"]
        out[p * ASLAB:(p + 1) * ASLAB, :] = part.T
    return out + bond


# revision 3
# speedup vs baseline: 1.3406x; 1.3406x over previous
"""ChemConv Bass kernel for 8 TRN2 NeuronCores.

Reference math:
    node_connection[a,f,i] = sum_n conn[a,n,f] * x[n,i]
    bond_score[a,o,f]      = sum_i node_connection[a,f,i] * pf[o,f,i]
    out[a,o] = sum_f bond_score[a,o,f]*bf[o,f,0] + sum_{f,c} bp[a,f,c]*bf[o,f,1+c]
collapses to
    out[a,o] = sum_{n,f} conn[a,n,f] * Y[n,f,o] + bond[a,o]
    Y[n,f,o] = sum_i x[n,i] * W[o,f,i],   W = pf * bf[:,:,0:1]

Device does the single large contraction (conn is 201MB, the memory-bound
stream); Y (3MB) and the bond term (0.5MB) are precomputed on the host and
the bond term is added host-side after the gather.

Sharding: cores 2p and 2p+1 share atom slab p (512 atoms); core 2p
contracts n-blocks 0..7, core 2p+1 blocks 8..15 (k-split).  The host sums
each pair's partial outputs.  This shape lets every matmul be the
max-size instruction [K=128]x[M=64]x[N=512] (PSUM bank caps N*4B at 2KB),
so each core runs only 96 matmuls in one PSUM accumulation group - the
kernel is bound by per-instruction cost, so instruction count is the
metric that matters (~103/core vs ~420 for the naive schedule).

conn is cast to bf16 host-side (halves HBM bytes; measured output rel-err
~2e-3 vs the 2e-2 gate) and laid out per-partition-contiguous
[128, 96*512] so each DMA slice moves 24KB-contiguous runs per partition
instead of 1KB strided pieces.
"""

import numpy as np
import ml_dtypes

import concourse.bass as bass
import concourse.tile as tile
from concourse import bacc, mybir
from concourse.bass_utils import run_bass_kernel_spmd

A = 2048
IN_DEPTH = 64
OUT_DEPTH = 64
F = 12
NCORES = 8
NPAIR = NCORES // 2
ASLAB = A // NPAIR        # 512 atoms per core pair
KP = 128
NBLK = A // KP            # 16 n-blocks total
NBH = NBLK // 2           # 8 n-blocks per core (k-split)
JC = NBH * F              # 96 contraction chunks per core
YW = F * OUT_DEPTH        # 768

BF16 = mybir.dt.bfloat16
F32 = mybir.dt.float32
NPBF16 = ml_dtypes.bfloat16

_cache = {}


def _build_nc(repeat=1, slices=2, conn_kind="ExternalInput"):
    assert JC % slices == 0
    CPS = JC // slices
    CW = CPS * ASLAB

    nc = bacc.Bacc("TRN2", target_bir_lowering=False, debug=False)

    connb = nc.dram_tensor("connb", [KP, JC * ASLAB], BF16, kind=conn_kind).ap()
    yb = nc.dram_tensor("yb", [KP, JC * OUT_DEPTH], BF16,
                        kind="ExternalInput").ap()
    out_t = nc.dram_tensor("out_t", [OUT_DEPTH, ASLAB], F32,
                           kind="ExternalOutput").ap()

    with tile.TileContext(nc) as tc:
        with (
            tc.tile_pool(name="const", bufs=1) as cpool,
            tc.tile_pool(name="conn", bufs=1) as connpool,
            tc.tile_pool(name="osb", bufs=2) as opool,
            tc.tile_pool(name="acc", bufs=2, space="PSUM") as apool,
        ):
            # yb on the ACT ring so the SP ring belongs to the conn stream
            y_sb = cpool.tile([KP, JC * OUT_DEPTH], BF16)
            nc.scalar.dma_start(y_sb[:], yb[:])

            for rep in range(repeat):
                csb = connpool.tile([KP, JC * ASLAB], BF16, tag="conn",
                                    name=f"conn_{rep}")
                for s in range(slices):
                    nc.sync.dma_start(csb[:, s * CW:(s + 1) * CW],
                                      connb[:, s * CW:(s + 1) * CW])

                acc = apool.tile([OUT_DEPTH, ASLAB], F32, tag="acc")
                for j in range(JC):
                    nc.tensor.matmul(
                        acc[:],
                        y_sb[:, j * OUT_DEPTH:(j + 1) * OUT_DEPTH],
                        csb[:, j * ASLAB:(j + 1) * ASLAB],
                        start=(j == 0), stop=(j == JC - 1),
                    )

                out_sb = opool.tile([OUT_DEPTH, ASLAB], F32, tag="osb")
                nc.vector.tensor_copy(out_sb[:], acc[:])
                nc.sync.dma_start(out_t[:], out_sb[:])

    nc.compile()
    return nc


def _prep(node_property_tensor, connectivity_tensor, bond_property_tensor,
          property_filters, bond_filters):
    x = np.asarray(node_property_tensor, dtype=np.float32)
    conn = np.asarray(connectivity_tensor, dtype=np.float32)
    pf = np.asarray(property_filters, dtype=np.float32)
    bf = np.asarray(bond_filters, dtype=np.float32)

    W = pf * bf[:, :, 0:1]                                   # (O, F, I)
    wr = W.transpose(2, 1, 0).reshape(IN_DEPTH, YW)          # (I, F*O)
    Y = (x @ wr).astype(NPBF16)                              # (A, F*O)

    conn_bf = conn.astype(NPBF16)                            # (A, A, F)

    in_maps = []
    for c in range(NCORES):
        p, half = divmod(c, 2)
        a0 = p * ASLAB
        nb0 = half * NBH
        # conn[a, n, f] -> [p=n%128, nb_local, f, a] -> [128, JC*ASLAB]
        sub = conn_bf[a0:a0 + ASLAB, nb0 * KP:(nb0 + NBH) * KP, :]
        connH = np.ascontiguousarray(
            sub.reshape(ASLAB, NBH, KP, F).transpose(2, 1, 3, 0)
        ).reshape(KP, JC * ASLAB)
        # Y[n, f*64+o] -> [p, nb_local, f, o] -> [128, JC*64]
        ybH = np.ascontiguousarray(
            Y[nb0 * KP:(nb0 + NBH) * KP].reshape(NBH, KP, F, OUT_DEPTH)
            .transpose(1, 0, 2, 3)
        ).reshape(KP, JC * OUT_DEPTH)
        in_maps.append({"connb": connH, "yb": ybH})
    return in_maps


def kernel(node_property_tensor, connectivity_tensor, bond_property_tensor,
           property_filters, bond_filters):
    bp = np.asarray(bond_property_tensor, dtype=np.float32)
    bf = np.asarray(bond_filters, dtype=np.float32)

    in_maps = _prep(node_property_tensor, connectivity_tensor,
                    bond_property_tensor, property_filters, bond_filters)

    if "nc" not in _cache:
        _cache["nc"] = _build_nc()
    nc = _cache["nc"]

    res = run_bass_kernel_spmd(nc, in_maps, core_ids=list(range(NCORES)))

    # host: sum the two k-half partials per slab, add the (tiny) bond term
    bond = np.einsum('afc,ofc->ao', bp, bf[:, :, 1:3]).astype(np.float32)
    out = np.empty((A, OUT_DEPTH), dtype=np.float32)
    for p in range(NPAIR):
        part = res.results[2 * p]["out_t"] + res.results[2 * p + 1]["out_t"]
        out[p * ASLAB:(p + 1) * ASLAB, :] = part.T
    return out + bond
